# revision 23
# baseline (speedup 1.0000x reference)
"""Trainium2 Bass kernel for masked-LSTM sentence classifier (nn_ABSA_Lstm).

Data-parallel over 8 NeuronCores, 128 sentences per core.

v4: packed contraction. The per-step gate matmul contracts x (301 rows incl.
bias-ones) and h (300 rows) in ONE 5-pass K=620 contraction instead of 3+3
K-tiles, by host-permuting the combined weight rows:

  pass0 = x^T[  0:128]          pass1 = x^T[128:256]
  pass2 = h^T[  0:128]          pass3 = h^T[128:256]
  pass4 = [ h^T[256:300] | 0*20 | x^T[256:300] | ones | 0*19 ]  (mixed tile)

The mixed tile costs nothing extra: emb is host-rearranged so the gathered/
xbar-transposed chunk2 lands with zeros in partitions 0:64 and the x-tail in
64:128; the per-step DVE copy of the third h-transpose chunk overwrites
partitions 0:44 in place.  20 matmuls of N=300 per step (vs 24) plus 3 PE
transposes; h is transposed in three chunks (128/128/44) with three DVE
copies so the three h-passes pipeline behind the elementwise chain.

Everything else follows v3: gate order [f|i|g|o], per-gate PSUM, x-prefill
of the two pure-x passes as PE filler, indirect-DMA gather pipeline with
xbar transposes, PE warm-up, masked output via hout += delta_t * h_t, and
the multi-wait splitting post-pass.
"""

import sys

for _p in ("/opt/trn_rl_repo", "/root/.axon_site/_ro/trn_rl_repo"):
    if _p not in sys.path:
        sys.path.append(_p)

import numpy as np
import ml_dtypes

from concourse import bass, mybir
import concourse.tile as tile
from concourse.bass import IndirectOffsetOnAxis
from concourse.bass_utils import run_bass_kernel_spmd
from concourse.masks import make_identity

B, T, V, D, H, C = 1024, 80, 50000, 300, 300, 3
G = 4 * H            # 1200 gate columns, order [f | i | g | o]
N_CORES = 8
BC = B // N_CORES    # 128 sentences per core
P = 128

F32 = mybir.dt.float32
BF16 = mybir.dt.bfloat16
I32 = mybir.dt.int32

# gate column ranges in the permuted weights
NF, NI, NG, NO = (0, H), (H, 2 * H), (2 * H, 3 * H), (3 * H, G)
GATES = (NF, NI, NG, NO)


def _sync_wait(sem_id, value):
    import bass_rust
    return bass_rust.SyncWait(
        sync_type="semaphore", id=sem_id, ant_name=f"splitsem_{sem_id}",
        wait_mode="sem-ge-imm", wait_value=value, wait_reg=None,
    )


def _sync_update(sem_id, mode, value):
    import bass_rust
    return bass_rust.SyncUpdate(
        sync_type="semaphore", id=sem_id, ant_name=f"splitsem_{sem_id}",
        update_mode=mode, update_value=value, update_reg=None,
    )


def _split_multi_waits(nc, spare_sem_ids):
    """walrus caps sync waits per instruction at 1 for every struct we hit.

    Engine instructions: spill excess waits onto single-wait NoOps placed
    just before, on the same engine (engine streams are in-order).

    DMA/queue instructions: a preceding engine NoOp may not order the DGE
    ring, so the spill NoOps perform ALL the original waits and the last one
    increments a dedicated semaphore; the DMA's single wait becomes that
    semaphore. Each such semaphore is decremented back to 0 at the kernel
    tail so repeated NEFF executions stay correct."""
    f = nc.m.functions[0]
    spare = list(spare_sem_ids)
    eng_sem = {}     # engine -> sem id (one per issuing engine, in-order stream)
    eng_count = {}   # engine -> number of increments so far
    for blk in f.blocks:
        out = []
        for ins in blk.instructions:
            si = ins.sync_info
            waits = list(si.on_wait) if si and si.on_wait else []
            if len(waits) <= 1:
                out.append(ins)
                continue
            tname = type(ins).__name__
            is_dma = ("DMA" in tname or "TensorLoad" in tname
                      or "TensorSave" in tname)
            if is_dma:
                eng = ins.engine
                if eng not in eng_sem:
                    eng_sem[eng] = spare.pop()
                    eng_count[eng] = 0
                sid = eng_sem[eng]
                eng_count[eng] += 1
                target = eng_count[eng]
                for j, w in enumerate(waits):
                    nop = mybir.InstNoOp(name=f"nop-dsplit-{nc.next_id()}")
                    nop.engine = eng
                    upd = [_sync_update(sid, "sem-inc", 1)] if j == len(waits) - 1 else []
                    nop.sync_info = mybir.SyncInfo(on_wait=[w], on_update=upd)
                    out.append(nop)
                si.on_wait = [_sync_wait(sid, target)]
            else:
                for w in waits[:-1]:
                    nop = mybir.InstNoOp(name=f"nop-split-{nc.next_id()}")
                    nop.engine = ins.engine
                    nop.sync_info = mybir.SyncInfo(on_wait=[w], on_update=[])
                    out.append(nop)
                si.on_wait = waits[-1:]
            out.append(ins)
        blk.instructions = out
    # tail: restore spilled-DMA semaphores to 0 for repeat executions
    if eng_sem:
        last_blk = f.blocks[-1]
        tail = list(last_blk.instructions)
        for eng, sid in eng_sem.items():
            nop = mybir.InstNoOp(name=f"nop-dclear-{nc.next_id()}")
            nop.engine = mybir.EngineType.SP
            nop.sync_info = mybir.SyncInfo(
                on_wait=[], on_update=[_sync_update(sid, "sem-sub-imm", eng_count[eng])]
            )
            tail.append(nop)
        last_blk.instructions = tail
    return sum(eng_count.values())


def build(t_steps=T, split_waits=True):
    nc = bass.Bass()
    spare_sems = [nc.alloc_semaphore(f"splitspare{i}") for i in range(48)]

    sent_e = nc.declare_dram_parameter("sent", [BC, T], I32, isOutput=False)
    lensm1_e = nc.declare_dram_parameter("lensm1", [BC, 1], F32, isOutput=False)
    # emb host-rearranged to [V, 384]:
    #   cols 0:256   = emb[:, 0:256]
    #   cols 256:320 = 0          (chunk2 partitions 0:64 after transpose)
    #   cols 320:364 = emb[:, 256:300]
    #   col  364     = 1.0        (bias ones-row)
    #   cols 365:384 = 0
    emb_e = nc.declare_dram_parameter("emb", [V, 3 * P], BF16, isOutput=False)
    # combined per-pass weights [128, 5, 1200]
    wcomb_e = nc.declare_dram_parameter("wcomb", [5 * P, G], BF16, isOutput=False)
    wout_e = nc.declare_dram_parameter("wout", [3 * P, C], BF16, isOutput=False)
    bout_e = nc.declare_dram_parameter("bout", [1, C], F32, isOutput=False)
    arange_e = nc.declare_dram_parameter("arange", [1, T], F32, isOutput=False)
    out_e = nc.declare_dram_parameter("out", [BC, C], F32, isOutput=True)

    with tile.TileContext(nc) as tc:
        with (
            tc.tile_pool(name="const", bufs=1) as const,
            tc.tile_pool(name="wpool", bufs=1) as wpool,
            tc.tile_pool(name="xtp", bufs=1) as xtp,
            tc.tile_pool(name="work", bufs=3) as work,
            tc.tile_pool(name="psum", bufs=2, space="PSUM") as psum,
        ):
            # ---- sent first: the gather pipeline depends on it ----
            sent_sb = const.tile([BC, T], I32)
            nc.sync.dma_start(out=sent_sb[:], in_=sent_e[:])

            ident = const.tile([P, P], F32)
            identb = const.tile([P, P], BF16)
            make_identity(nc, ident)
            nc.vector.tensor_copy(out=identb[:], in_=ident[:])

            # warm the PE HAM clock gate (~3.4us of sustained activity flips
            # K=4/8 -> 8/8) with junk matmuls while the PE is otherwise idle
            # waiting for the first gathers, so the quad-0 transposes, first
            # x-matmuls, and steps 0-1 all run at full clock
            warm = psum.tile([P, P], F32, name="warm", tag="trp",
                             bufs=1)
            for _ in range(36):
                nc.tensor.matmul(out=warm[:, 0:P], lhsT=identb[:],
                                 rhs=identb[:, 0:P], start=True, stop=True)

            # x staging: one standalone tile per group, written only by
            # its gather and read only by its transpose -- zero cross-step
            # dependencies, so both DMA queues free-run ahead of the scan.
            qsizes = []
            left = t_steps
            for s in (2, 2):
                if left > 0:
                    s = min(s, left)
                    qsizes.append(s)
                    left -= s
            while left > 0:
                s = min(4, left)
                qsizes.append(s)
                left -= s
            qof = []
            for qi, s in enumerate(qsizes):
                for o in range(s):
                    qof.append((qi, o))
            xts = [
                xtp.tile([P, 3 * s, P], BF16, name=f"xt{i}")
                for i, s in enumerate(qsizes)
            ]
            xpads = [
                xtp.tile([P, s * 3 * P], BF16, name=f"xpad{i}")
                for i, s in enumerate(qsizes)
            ]

            def xt_slice(t, k):
                qi, o = qof[t]
                return xts[qi][:, 3 * o + k, :]

            def prep_gather(t):
                qi, o = qof[t]
                nc.gpsimd.indirect_dma_start(
                    out=xpads[qi][:, o * 3 * P : (o + 1) * 3 * P],
                    out_offset=None, in_=emb_e[:],
                    in_offset=IndirectOffsetOnAxis(ap=sent_sb[:, t : t + 1], axis=0),
                )

            def prep_transpose(q):
                nc.sync.dma_start_transpose(
                    out=xts[q][:, :, :], in_=xpads[q][:]
                )

            # interleave gather/transpose emission so the static schedule
            # pipelines the two queues; quad 0 is transposed on the PE (idle
            # during the prologue), skipping the xbar queue's first-hop
            # latency so step 0's x-matmuls start several us earlier
            QT = 4
            for u in range(min(QT, t_steps)):
                prep_gather(u)
                qi, o = qof[u]
                if o == qsizes[qi] - 1 and qi > 0:
                    prep_transpose(qi)
            for o in range(qsizes[0]):
                trp0 = psum.tile([P, 3 * P], BF16, name="trpq0", tag="trp",
                                 bufs=1)
                for k in range(3):
                    nc.tensor.transpose(
                        out=trp0[:, k * P : (k + 1) * P],
                        in_=xpads[0][:, (o * 3 + k) * P : (o * 3 + k + 1) * P],
                        identity=identb[:],
                    )
                nc.vector.tensor_copy(
                    out=xts[0][:, 3 * o : 3 * o + 3, :], in_=trp0[:])
            # weights on the scalar DMA queue (one DMA per tensor), in
            # parallel with the gathers and transposes
            wc_sb = wpool.tile([P, 5, G], BF16, name="wc_sb")
            wc_src = bass.AP(tensor=wcomb_e, offset=0,
                             ap=[[G, P], [P * G, 5], [1, G]])
            nc.scalar.dma_start(out=wc_sb[:], in_=wc_src)
            wc_t = [wc_sb[:, p, :] for p in range(5)]
            wout_sb = wpool.tile([P, 3, C], BF16, name="wout_sb")
            wout_src = bass.AP(tensor=wout_e, offset=0,
                               ap=[[C, P], [P * C, 3], [1, C]])
            nc.scalar.dma_start(out=wout_sb[:], in_=wout_src)
            wout_t = [wout_sb[:, k, :] for k in range(3)]

            lensm1 = const.tile([BC, 1], F32)
            nc.scalar.dma_start(out=lensm1[:], in_=lensm1_e[:])

            arange_sb = const.tile([BC, T], F32)
            arange_bcast = bass.AP(
                tensor=arange_e, offset=0, ap=[[0, BC], [1, T]]
            )
            nc.gpsimd.dma_start(out=arange_sb[:], in_=arange_bcast)

            # delta[b,t] = (t == lens[b]-1), as f32
            delta = const.tile([BC, T], F32)
            nc.vector.tensor_scalar(
                out=delta[:], in0=arange_sb[:], scalar1=lensm1[:, 0:1],
                scalar2=None, op0=mybir.AluOpType.is_equal,
            )

            bout_sb = const.tile([BC, C], F32)
            bout_bcast = bass.AP(
                tensor=bout_e, offset=0, ap=[[0, BC], [1, C]]
            )
            nc.gpsimd.dma_start(out=bout_sb[:], in_=bout_bcast)

            for u in range(QT, t_steps):
                prep_gather(u)
                qi, o = qof[u]
                if o == qsizes[qi] - 1:
                    prep_transpose(qi)

            # preload the sigmoid/tanh ACT table set during the prologue
            actpre = const.tile([BC, 1], BF16)
            nc.scalar.activation(
                out=actpre[:, 0:1], in_=lensm1[:, 0:1],
                func=mybir.ActivationFunctionType.Sigmoid,
            )

            # ACT filler scratch: keeps the scalar engine busy through its
            # per-step idle window so sigmoid-f issues without an idle->wake
            # penalty
            actfill = const.tile([BC, 700], F32)
            nc.vector.memset(actfill[:], 0.0)

            # ---- scan state ----
            hout = const.tile([BC, H], F32)
            nc.vector.memset(hout[:], 0.0)

            # h^T rings for passes 2 and 3 (chunks 0:128 and 128:256)
            htA = [const.tile([P, P], BF16, name=f"htA{i}") for i in range(2)]
            htB = [const.tile([P, P], BF16, name=f"htB{i}") for i in range(2)]

            # padded h_new for the output projection epilogue
            houtb = const.tile([BC, 3 * P], BF16, name="houtb")
            nc.vector.memset(houtb[:, D : 3 * P], 0.0)

            # single-buffered per-gate PSUM (4 banks) + 3 transpose banks
            psf = psum.tile([BC, H], F32, name="psff", tag="psf", bufs=1)
            psi = psum.tile([BC, H], F32, name="psii", tag="psi", bufs=1)
            psg = psum.tile([BC, H], F32, name="psgg", tag="psg", bufs=1)
            pso = psum.tile([BC, H], F32, name="psoo", tag="pso", bufs=1)
            PS = {"f": psf, "i": psi, "g": psg, "o": pso}
            NR = {"f": NF, "i": NI, "g": NG, "o": NO}

            def x_pass(t, gate, p, start):
                n0, n1 = NR[gate]
                nc.tensor.matmul(
                    out=PS[gate][:, 0:H], lhsT=xt_slice(t, p),
                    rhs=wc_t[p][:, n0:n1], start=start, stop=False,
                )

            def h_pass(gate, lhsT, p, stop):
                n0, n1 = NR[gate]
                nc.tensor.matmul(
                    out=PS[gate][:, 0:H], lhsT=lhsT,
                    rhs=wc_t[p][:, n0:n1], start=False, stop=stop,
                )

            for gate in "figo":
                x_pass(0, gate, 0, start=True)
                x_pass(0, gate, 1, start=False)

            from bass_rust import add_dep_helper

            c_prev = None
            h_prev = None
            fill_prev = None
            have_h = False
            for t in range(t_steps):
                last = t + 1 >= t_steps
                ra, rb = htA[t % 2], htB[t % 2]     # written at end of step t-1
                s4 = xt_slice(t, 2)                 # mixed tile (h-tail in 0:44)
                # h-passes gate-major so each gate's stop lands early; the
                # o-gate x-prefill pair (deferred from step t-1) slots in
                # right after the f-group so it can't delay sigmoid-f
                if have_h:
                    first_gate = True
                    for gate in "figo":
                        h_pass(gate, ra[:], 2, stop=False)
                        h_pass(gate, rb[:], 3, stop=False)
                        h_pass(gate, s4, 4, stop=True)
                        if first_gate:
                            first_gate = False
                            x_pass(t, "o", 0, start=True)
                            x_pass(t, "o", 1, start=False)
                else:
                    # step 0: x-tail still contributes through the mixed tile
                    # (partitions 0:64 are zeros from the rearranged emb)
                    for gate in "figo":
                        h_pass(gate, s4, 4, stop=True)

                # minimal-loop tail: F/I/G/O unchunked; t1/c and tanh_c
                # chunked at [0:128]/[128:300] to match the transpose chunks
                tf = work.tile([BC, H], BF16, name="tf", tag="tf")
                ti = work.tile([BC, H], BF16, name="ti", tag="ti")
                tg = work.tile([BC, H], BF16, name="tg", tag="tg")
                to = work.tile([BC, H], BF16, name="to", tag="to")
                t2_ = work.tile([BC, H], BF16, name="t2_", tag="t2_")
                t1_ = work.tile([BC, H], BF16, name="t1_", tag="t1_")
                c_new = work.tile([BC, H], BF16, name="c_new", tag="c_new")
                tc_ = work.tile([BC, H], BF16, name="tc_", tag="tc_")
                h_new = work.tile([BC, H], BF16, name="h_new", tag="h_new")

                SIG = mybir.ActivationFunctionType.Sigmoid
                TANH = mybir.ActivationFunctionType.Tanh

                f_act = nc.scalar.activation(out=tf[:], in_=psf[:], func=SIG)
                if fill_prev is not None:
                    # keep ACT FIFO order: filler(t-1) then sigmoid-f(t)
                    add_dep_helper(f_act.ins, fill_prev.ins, sync=False,
                                   reason="f after filler")
                if not last:
                    x_pass(t + 1, "f", 0, start=True)
                    x_pass(t + 1, "f", 1, start=False)
                nc.scalar.activation(out=ti[:], in_=psi[:], func=SIG)
                if not last:
                    x_pass(t + 1, "i", 0, start=True)
                    x_pass(t + 1, "i", 1, start=False)
                t2_ins = None
                if c_prev is not None:
                    t2_ins = nc.vector.tensor_mul(t2_[:], tf[:], c_prev[:])
                # previous step's capture fills the DVE window here so the
                # DVE is awake when tanh-g lands
                cap = None
                if h_prev is not None and t2_ins is not None:
                    cap = nc.vector.scalar_tensor_tensor(
                        out=hout[:], in0=h_prev[:, 0:H],
                        scalar=delta[:, t - 1 : t],
                        in1=hout[:], op0=mybir.AluOpType.mult,
                        op1=mybir.AluOpType.add,
                    )
                    add_dep_helper(cap.ins, t2_ins.ins, sync=False,
                                   reason="capture after t2")
                nc.scalar.activation(out=tg[:], in_=psg[:], func=TANH)
                if not last:
                    x_pass(t + 1, "g", 0, start=True)
                    x_pass(t + 1, "g", 1, start=False)
                # chunked t1 / c_new
                first_t1 = True
                for c0, c1 in ((0, P), (P, H)):
                    t1i = nc.vector.tensor_mul(t1_[:, c0:c1], ti[:, c0:c1],
                                               tg[:, c0:c1])
                    if first_t1 and cap is not None:
                        add_dep_helper(t1i.ins, cap.ins, sync=False,
                                       reason="t1 after capture")
                        first_t1 = False
                    if c_prev is not None:
                        nc.vector.tensor_add(c_new[:, c0:c1], t1_[:, c0:c1],
                                             t2_[:, c0:c1])
                    else:
                        nc.vector.tensor_copy(out=c_new[:, c0:c1],
                                              in_=t1_[:, c0:c1])
                nc.scalar.activation(out=to[:], in_=pso[:], func=SIG)
                nc.scalar.activation(out=tc_[:, 0:P], in_=c_new[:, 0:P],
                                     func=TANH)
                cb_act = nc.scalar.activation(out=tc_[:, P:H],
                                              in_=c_new[:, P:H], func=TANH)
                fill_prev = None

                last_copy = None
                if not last:
                    # h chunks -> separate PSUM transpose tiles -> copies
                    na, nb = htA[(t + 1) % 2], htB[(t + 1) % 2]
                    s4n = xt_slice(t + 1, 2)
                    trA = psum.tile([P, P], BF16, name="trA", tag="trpA",
                                    bufs=1)
                    trB = psum.tile([P, P], BF16, name="trB", tag="trpB",
                                    bufs=1)
                    trC = psum.tile([64, P], BF16, name="trC", tag="trpC",
                                    bufs=1)
                    nc.vector.tensor_mul(h_new[:, 0:P], to[:, 0:P], tc_[:, 0:P])
                    nc.vector.tensor_mul(h_new[:, P:2 * P], to[:, P:2 * P],
                                         tc_[:, P:2 * P])
                    nc.vector.tensor_mul(h_new[:, 2 * P:H], to[:, 2 * P:H],
                                         tc_[:, 2 * P:H])
                    nc.tensor.transpose(out=trA[:], in_=h_new[:, 0:P],
                                        identity=identb[:])
                    nc.tensor.transpose(out=trB[:], in_=h_new[:, P:2 * P],
                                        identity=identb[:])
                    nc.tensor.transpose(out=trC[0:H - 2 * P, :],
                                        in_=h_new[:, 2 * P:H],
                                        identity=identb[:])
                    nc.vector.tensor_copy(out=na[:], in_=trA[:])
                    # middle copy rides the scalar engine (it can read PSUM)
                    # so the DVE copy chain is 2 ops, not 3
                    cpb_act = nc.scalar.activation(
                        out=nb[:], in_=trB[:],
                        func=mybir.ActivationFunctionType.Copy,
                    )
                    last_copy = nc.vector.tensor_copy(
                        out=s4n[0:H - 2 * P, :],
                        in_=trC[0:H - 2 * P, :])
                    # ACT idle-window filler (see actfill above), pinned
                    # between the trB scalar-copy and sigmoid-f(t+1)
                    fill_prev = nc.scalar.activation(
                        out=actfill[:], in_=actfill[:],
                        func=mybir.ActivationFunctionType.Copy,
                    )
                    add_dep_helper(fill_prev.ins, cpb_act.ins, sync=False,
                                   reason="filler after trB copy")
                else:
                    nc.vector.tensor_mul(h_new[:, 0:P], to[:, 0:P], tc_[:, 0:P])
                    nc.vector.tensor_mul(h_new[:, P:2 * P], to[:, P:2 * P],
                                         tc_[:, P:2 * P])
                    nc.vector.tensor_mul(h_new[:, 2 * P:H], to[:, 2 * P:H],
                                         tc_[:, 2 * P:H])

                # hout += delta_t * h is deferred into the NEXT step's DVE
                # window (h_prev); the final step's capture happens after the
                # loop
                c_prev = c_new
                h_prev = h_new
                have_h = True

            # final step's capture
            nc.vector.scalar_tensor_tensor(
                out=hout[:], in0=h_prev[:, 0:H],
                scalar=delta[:, t_steps - 1 : t_steps],
                in1=hout[:], op0=mybir.AluOpType.mult,
                op1=mybir.AluOpType.add,
            )

            # ---- output projection (bf16, via the padded houtb tile)
            nc.vector.tensor_copy(out=houtb[:, 0:H], in_=hout[:])
            hot = work.tile([P, 3, P], BF16, name="hot")
            trpo = psum.tile([P, 3 * P], BF16, name="trpo", tag="trp", bufs=1)
            for k in range(3):
                nc.tensor.transpose(
                    out=trpo[:, k * P : (k + 1) * P],
                    in_=houtb[:, k * P : (k + 1) * P], identity=identb[:]
                )
            nc.vector.tensor_copy(out=hot[:, :, :], in_=trpo[:])
            po = psum.tile([P, P], F32, name="po", tag="pso", bufs=1)
            for k in range(3):
                nc.tensor.matmul(
                    out=po[:, 0:C],
                    lhsT=hot[:, k, :],
                    rhs=wout_t[k][:, :],
                    start=(k == 0),
                    stop=(k == 2),
                )
            logit = work.tile([BC, C], F32, name="logit")
            nc.vector.tensor_add(logit[:], po[:, 0:C], bout_sb[:])
            nc.sync.dma_start(out=out_e[:], in_=logit[:])

    if split_waits:
        _split_multi_waits(nc, [s.num for s in spare_sems])
    return nc


_NC_CACHE = {}


def _get_nc(t_steps=T):
    if t_steps not in _NC_CACHE:
        _NC_CACHE[t_steps] = build(t_steps)
    return _NC_CACHE[t_steps]


def make_in_maps(sent, lens, emb, Wx, Wh, b, Wout, bout):
    # permute gate columns [i|f|g|o] -> [f|i|g|o]
    perm = np.concatenate(
        [np.arange(300, 600), np.arange(0, 300), np.arange(600, 900),
         np.arange(900, 1200)]
    )
    Wxp = np.asarray(Wx, np.float32)[:, perm]
    Whp = np.asarray(Wh, np.float32)[:, perm]
    bp = np.asarray(b, np.float32)[perm]

    # combined 5-pass weights [5*128, 1200]
    wcomb = np.zeros((5 * P, G), np.float32)
    wcomb[0:P] = Wxp[0:P]
    wcomb[P:2 * P] = Wxp[P:2 * P]
    wcomb[2 * P:3 * P] = Whp[0:P]
    wcomb[3 * P:4 * P] = Whp[P:2 * P]
    # pass 4 mixed: rows 0:44 = Wh tail, 64:108 = Wx tail, 108 = bias
    wcomb[4 * P + 0 : 4 * P + (H - 2 * P)] = Whp[2 * P:H]
    wcomb[4 * P + 64 : 4 * P + 64 + (D - 2 * P)] = Wxp[2 * P:D]
    wcomb[4 * P + 64 + (D - 2 * P)] = bp
    wcomb = np.ascontiguousarray(wcomb.astype(ml_dtypes.bfloat16))

    # emb rearranged: cols 0:256 data, 256:320 zero, 320:364 tail data,
    # 364 ones, 365:384 zero
    embf = np.asarray(emb, np.float32)
    emb_pad = np.zeros((V, 3 * P), np.float32)
    emb_pad[:, 0:2 * P] = embf[:, 0:2 * P]
    emb_pad[:, 2 * P + 64 : 2 * P + 64 + (D - 2 * P)] = embf[:, 2 * P:D]
    emb_pad[:, 2 * P + 64 + (D - 2 * P)] = 1.0
    emb_pad = np.ascontiguousarray(emb_pad.astype(ml_dtypes.bfloat16))

    wout_pad = np.zeros((3 * P, C), np.float32)
    wout_pad[:H, :] = np.asarray(Wout, np.float32)
    wout = np.ascontiguousarray(wout_pad.astype(ml_dtypes.bfloat16))
    bout2 = np.asarray(bout, np.float32).reshape(1, C)
    arange = np.arange(T, dtype=np.float32).reshape(1, T)

    in_maps = []
    for i in range(N_CORES):
        sl = slice(i * BC, (i + 1) * BC)
        in_maps.append({
            "sent": np.ascontiguousarray(np.asarray(sent, np.int32)[sl]),
            "lensm1": (np.asarray(lens, np.int32)[sl] - 1).reshape(BC, 1).astype(np.float32),
            "emb": emb_pad,
            "wcomb": wcomb,
            "wout": wout,
            "bout": bout2,
            "arange": arange,
        })
    return in_maps


def kernel(sent, lens, emb, Wx, Wh, b, Wout, bout):
    nc = _get_nc(T)
    in_maps = make_in_maps(sent, lens, emb, Wx, Wh, b, Wout, bout)
    res = run_bass_kernel_spmd(nc, in_maps, core_ids=list(range(N_CORES)))
    out = np.concatenate(
        [res.results[i]["out"] for i in range(N_CORES)], axis=0
    )
    return out.astype(np.float32)


# revision 24
# speedup vs baseline: 1.0250x; 1.0250x over previous
"""Trainium2 Bass kernel for masked-LSTM sentence classifier (nn_ABSA_Lstm).

Data-parallel over 8 NeuronCores, 128 sentences per core.

v4: packed contraction. The per-step gate matmul contracts x (301 rows incl.
bias-ones) and h (300 rows) in ONE 5-pass K=620 contraction instead of 3+3
K-tiles, by host-permuting the combined weight rows:

  pass0 = x^T[  0:128]          pass1 = x^T[128:256]
  pass2 = h^T[  0:128]          pass3 = h^T[128:256]
  pass4 = [ h^T[256:300] | 0*20 | x^T[256:300] | ones | 0*19 ]  (mixed tile)

The mixed tile costs nothing extra: emb is host-rearranged so the gathered/
xbar-transposed chunk2 lands with zeros in partitions 0:64 and the x-tail in
64:128; the per-step DVE copy of the third h-transpose chunk overwrites
partitions 0:44 in place.  20 matmuls of N=300 per step (vs 24) plus 3 PE
transposes; h is transposed in three chunks (128/128/44) with three DVE
copies so the three h-passes pipeline behind the elementwise chain.

Everything else follows v3: gate order [f|i|g|o], per-gate PSUM, x-prefill
of the two pure-x passes as PE filler, indirect-DMA gather pipeline with
xbar transposes, PE warm-up, masked output via hout += delta_t * h_t, and
the multi-wait splitting post-pass.
"""

import sys

for _p in ("/opt/trn_rl_repo", "/root/.axon_site/_ro/trn_rl_repo"):
    if _p not in sys.path:
        sys.path.append(_p)

import numpy as np
import ml_dtypes

from concourse import bass, mybir
import concourse.tile as tile
from concourse.bass import IndirectOffsetOnAxis
from concourse.bass_utils import run_bass_kernel_spmd
from concourse.masks import make_identity

B, T, V, D, H, C = 1024, 80, 50000, 300, 300, 3
G = 4 * H            # 1200 gate columns, order [f | i | g | o]
N_CORES = 8
BC = B // N_CORES    # 128 sentences per core
P = 128

F32 = mybir.dt.float32
BF16 = mybir.dt.bfloat16
I32 = mybir.dt.int32

# gate column ranges in the permuted weights
NF, NI, NG, NO = (0, H), (H, 2 * H), (2 * H, 3 * H), (3 * H, G)
GATES = (NF, NI, NG, NO)


def _sync_wait(sem_id, value):
    import bass_rust
    return bass_rust.SyncWait(
        sync_type="semaphore", id=sem_id, ant_name=f"splitsem_{sem_id}",
        wait_mode="sem-ge-imm", wait_value=value, wait_reg=None,
    )


def _sync_update(sem_id, mode, value):
    import bass_rust
    return bass_rust.SyncUpdate(
        sync_type="semaphore", id=sem_id, ant_name=f"splitsem_{sem_id}",
        update_mode=mode, update_value=value, update_reg=None,
    )


def _split_multi_waits(nc, spare_sem_ids):
    """walrus caps sync waits per instruction at 1 for every struct we hit.

    Engine instructions: spill excess waits onto single-wait NoOps placed
    just before, on the same engine (engine streams are in-order).

    DMA/queue instructions: a preceding engine NoOp may not order the DGE
    ring, so the spill NoOps perform ALL the original waits and the last one
    increments a dedicated semaphore; the DMA's single wait becomes that
    semaphore. Each such semaphore is decremented back to 0 at the kernel
    tail so repeated NEFF executions stay correct."""
    f = nc.m.functions[0]
    spare = list(spare_sem_ids)
    eng_sem = {}     # engine -> sem id (one per issuing engine, in-order stream)
    eng_count = {}   # engine -> number of increments so far
    for blk in f.blocks:
        out = []
        for ins in blk.instructions:
            si = ins.sync_info
            waits = list(si.on_wait) if si and si.on_wait else []
            if len(waits) <= 1:
                out.append(ins)
                continue
            tname = type(ins).__name__
            is_dma = ("DMA" in tname or "TensorLoad" in tname
                      or "TensorSave" in tname)
            if is_dma:
                eng = ins.engine
                if eng not in eng_sem:
                    eng_sem[eng] = spare.pop()
                    eng_count[eng] = 0
                sid = eng_sem[eng]
                eng_count[eng] += 1
                target = eng_count[eng]
                for j, w in enumerate(waits):
                    nop = mybir.InstNoOp(name=f"nop-dsplit-{nc.next_id()}")
                    nop.engine = eng
                    upd = [_sync_update(sid, "sem-inc", 1)] if j == len(waits) - 1 else []
                    nop.sync_info = mybir.SyncInfo(on_wait=[w], on_update=upd)
                    out.append(nop)
                si.on_wait = [_sync_wait(sid, target)]
            else:
                for w in waits[:-1]:
                    nop = mybir.InstNoOp(name=f"nop-split-{nc.next_id()}")
                    nop.engine = ins.engine
                    nop.sync_info = mybir.SyncInfo(on_wait=[w], on_update=[])
                    out.append(nop)
                si.on_wait = waits[-1:]
            out.append(ins)
        blk.instructions = out
    # tail: restore spilled-DMA semaphores to 0 for repeat executions
    if eng_sem:
        last_blk = f.blocks[-1]
        tail = list(last_blk.instructions)
        for eng, sid in eng_sem.items():
            nop = mybir.InstNoOp(name=f"nop-dclear-{nc.next_id()}")
            nop.engine = mybir.EngineType.SP
            nop.sync_info = mybir.SyncInfo(
                on_wait=[], on_update=[_sync_update(sid, "sem-sub-imm", eng_count[eng])]
            )
            tail.append(nop)
        last_blk.instructions = tail
    return sum(eng_count.values())


def build(t_steps=T, split_waits=True):
    nc = bass.Bass()
    spare_sems = [nc.alloc_semaphore(f"splitspare{i}") for i in range(48)]

    sent_e = nc.declare_dram_parameter("sent", [BC, T], I32, isOutput=False)
    lensm1_e = nc.declare_dram_parameter("lensm1", [BC, 1], F32, isOutput=False)
    # emb host-rearranged to [V, 384]:
    #   cols 0:256   = emb[:, 0:256]
    #   cols 256:320 = 0          (chunk2 partitions 0:64 after transpose)
    #   cols 320:364 = emb[:, 256:300]
    #   col  364     = 1.0        (bias ones-row)
    #   cols 365:384 = 0
    emb_e = nc.declare_dram_parameter("emb", [V, 3 * P], BF16, isOutput=False)
    # combined per-pass weights [128, 5, 1200]
    wcomb_e = nc.declare_dram_parameter("wcomb", [5 * P, G], BF16, isOutput=False)
    wout_e = nc.declare_dram_parameter("wout", [3 * P, C], BF16, isOutput=False)
    bout_e = nc.declare_dram_parameter("bout", [1, C], F32, isOutput=False)
    arange_e = nc.declare_dram_parameter("arange", [1, T], F32, isOutput=False)
    out_e = nc.declare_dram_parameter("out", [BC, C], F32, isOutput=True)

    with tile.TileContext(nc) as tc:
        with (
            tc.tile_pool(name="const", bufs=1) as const,
            tc.tile_pool(name="wpool", bufs=1) as wpool,
            tc.tile_pool(name="xtp", bufs=1) as xtp,
            tc.tile_pool(name="work", bufs=3) as work,
            tc.tile_pool(name="psum", bufs=2, space="PSUM") as psum,
        ):
            # ---- sent first: the gather pipeline depends on it ----
            sent_sb = const.tile([BC, T], I32)
            nc.sync.dma_start(out=sent_sb[:], in_=sent_e[:])

            ident = const.tile([P, P], F32)
            identb = const.tile([P, P], BF16)
            make_identity(nc, ident)
            nc.vector.tensor_copy(out=identb[:], in_=ident[:])

            # warm the PE HAM clock gate (~3.4us of sustained activity flips
            # K=4/8 -> 8/8) with junk matmuls while the PE is otherwise idle
            # waiting for the first gathers, so the quad-0 transposes, first
            # x-matmuls, and steps 0-1 all run at full clock
            warm = psum.tile([P, P], F32, name="warm", tag="trp",
                             bufs=1)
            for _ in range(36):
                nc.tensor.matmul(out=warm[:, 0:P], lhsT=identb[:],
                                 rhs=identb[:, 0:P], start=True, stop=True)

            # x staging: one standalone tile per group, written only by
            # its gather and read only by its transpose -- zero cross-step
            # dependencies, so both DMA queues free-run ahead of the scan.
            qsizes = []
            left = t_steps
            for s in (2, 2):
                if left > 0:
                    s = min(s, left)
                    qsizes.append(s)
                    left -= s
            while left > 0:
                s = min(4, left)
                qsizes.append(s)
                left -= s
            qof = []
            for qi, s in enumerate(qsizes):
                for o in range(s):
                    qof.append((qi, o))
            xts = [
                xtp.tile([P, 3 * s, P], BF16, name=f"xt{i}")
                for i, s in enumerate(qsizes)
            ]
            xpads = [
                xtp.tile([P, s * 3 * P], BF16, name=f"xpad{i}")
                for i, s in enumerate(qsizes)
            ]

            def xt_slice(t, k):
                qi, o = qof[t]
                return xts[qi][:, 3 * o + k, :]

            def prep_gather(t):
                qi, o = qof[t]
                nc.gpsimd.indirect_dma_start(
                    out=xpads[qi][:, o * 3 * P : (o + 1) * 3 * P],
                    out_offset=None, in_=emb_e[:],
                    in_offset=IndirectOffsetOnAxis(ap=sent_sb[:, t : t + 1], axis=0),
                )

            def prep_transpose(q):
                nc.sync.dma_start_transpose(
                    out=xts[q][:, :, :], in_=xpads[q][:]
                )

            # interleave gather/transpose emission so the static schedule
            # pipelines the two queues; quad 0 is transposed on the PE (idle
            # during the prologue), skipping the xbar queue's first-hop
            # latency so step 0's x-matmuls start several us earlier
            QT = 4
            for u in range(min(QT, t_steps)):
                prep_gather(u)
                qi, o = qof[u]
                if o == qsizes[qi] - 1 and qi > 0:
                    prep_transpose(qi)
            for o in range(qsizes[0]):
                trp0 = psum.tile([P, 3 * P], BF16, name="trpq0", tag="trp",
                                 bufs=1)
                for k in range(3):
                    nc.tensor.transpose(
                        out=trp0[:, k * P : (k + 1) * P],
                        in_=xpads[0][:, (o * 3 + k) * P : (o * 3 + k + 1) * P],
                        identity=identb[:],
                    )
                nc.vector.tensor_copy(
                    out=xts[0][:, 3 * o : 3 * o + 3, :], in_=trp0[:])
            # weights on the scalar DMA queue (one DMA per tensor), in
            # parallel with the gathers and transposes
            wc_sb = wpool.tile([P, 5, G], BF16, name="wc_sb")
            wc_src = bass.AP(tensor=wcomb_e, offset=0,
                             ap=[[G, P], [P * G, 5], [1, G]])
            nc.scalar.dma_start(out=wc_sb[:], in_=wc_src)
            wc_t = [wc_sb[:, p, :] for p in range(5)]
            wout_sb = wpool.tile([P, 3, C], BF16, name="wout_sb")
            wout_src = bass.AP(tensor=wout_e, offset=0,
                               ap=[[C, P], [P * C, 3], [1, C]])
            nc.scalar.dma_start(out=wout_sb[:], in_=wout_src)
            wout_t = [wout_sb[:, k, :] for k in range(3)]

            lensm1 = const.tile([BC, 1], F32)
            nc.scalar.dma_start(out=lensm1[:], in_=lensm1_e[:])

            arange_sb = const.tile([BC, T], F32)
            arange_bcast = bass.AP(
                tensor=arange_e, offset=0, ap=[[0, BC], [1, T]]
            )
            nc.gpsimd.dma_start(out=arange_sb[:], in_=arange_bcast)

            # delta[b,t] = (t == lens[b]-1), as f32
            delta = const.tile([BC, T], F32)
            nc.vector.tensor_scalar(
                out=delta[:], in0=arange_sb[:], scalar1=lensm1[:, 0:1],
                scalar2=None, op0=mybir.AluOpType.is_equal,
            )

            bout_sb = const.tile([BC, C], F32)
            bout_bcast = bass.AP(
                tensor=bout_e, offset=0, ap=[[0, BC], [1, C]]
            )
            nc.gpsimd.dma_start(out=bout_sb[:], in_=bout_bcast)

            for u in range(QT, t_steps):
                prep_gather(u)
                qi, o = qof[u]
                if o == qsizes[qi] - 1:
                    prep_transpose(qi)

            # preload the sigmoid/tanh ACT table set during the prologue
            actpre = const.tile([BC, 1], BF16)
            nc.scalar.activation(
                out=actpre[:, 0:1], in_=lensm1[:, 0:1],
                func=mybir.ActivationFunctionType.Sigmoid,
            )

            # ACT filler scratch: keeps the scalar engine busy through its
            # per-step idle window so sigmoid-f issues without an idle->wake
            # penalty
            actfill = const.tile([BC, 300], F32)
            nc.vector.memset(actfill[:], 0.0)

            # ---- scan state ----
            hout = const.tile([BC, H], F32)
            nc.vector.memset(hout[:], 0.0)

            # h^T rings for passes 2 and 3 (chunks 0:128 and 128:256)
            htA = [const.tile([P, P], BF16, name=f"htA{i}") for i in range(2)]
            htB = [const.tile([P, P], BF16, name=f"htB{i}") for i in range(2)]

            # padded h_new for the output projection epilogue
            houtb = const.tile([BC, 3 * P], BF16, name="houtb")
            nc.vector.memset(houtb[:, D : 3 * P], 0.0)

            # single-buffered per-gate PSUM (4 banks) + 3 transpose banks
            psf = psum.tile([BC, H], F32, name="psff", tag="psf", bufs=1)
            psi = psum.tile([BC, H], F32, name="psii", tag="psi", bufs=1)
            psg = psum.tile([BC, H], F32, name="psgg", tag="psg", bufs=1)
            pso = psum.tile([BC, H], F32, name="psoo", tag="pso", bufs=1)
            PS = {"f": psf, "i": psi, "g": psg, "o": pso}
            NR = {"f": NF, "i": NI, "g": NG, "o": NO}

            def x_pass(t, gate, p, start):
                n0, n1 = NR[gate]
                nc.tensor.matmul(
                    out=PS[gate][:, 0:H], lhsT=xt_slice(t, p),
                    rhs=wc_t[p][:, n0:n1], start=start, stop=False,
                )

            def h_pass(gate, lhsT, p, stop):
                n0, n1 = NR[gate]
                nc.tensor.matmul(
                    out=PS[gate][:, 0:H], lhsT=lhsT,
                    rhs=wc_t[p][:, n0:n1], start=False, stop=stop,
                )

            for gate in "figo":
                x_pass(0, gate, 0, start=True)
                x_pass(0, gate, 1, start=False)

            from bass_rust import add_dep_helper

            c_prev = None
            h_prev = None
            fill_prev = None
            have_h = False
            for t in range(t_steps):
                last = t + 1 >= t_steps
                ra, rb = htA[t % 2], htB[t % 2]     # written at end of step t-1
                s4 = xt_slice(t, 2)                 # mixed tile (h-tail in 0:44)
                # h-passes gate-major so each gate's stop lands early; the
                # o-gate x-prefill pair (deferred from step t-1) slots in
                # right after the f-group so it can't delay sigmoid-f
                if have_h:
                    first_gate = True
                    for gate in "figo":
                        h_pass(gate, ra[:], 2, stop=False)
                        h_pass(gate, rb[:], 3, stop=False)
                        h_pass(gate, s4, 4, stop=True)
                        if first_gate:
                            first_gate = False
                            x_pass(t, "o", 0, start=True)
                            x_pass(t, "o", 1, start=False)
                else:
                    # step 0: x-tail still contributes through the mixed tile
                    # (partitions 0:64 are zeros from the rearranged emb)
                    for gate in "figo":
                        h_pass(gate, s4, 4, stop=True)

                # minimal-loop tail: F/I/G/O unchunked; t1/c and tanh_c
                # chunked at [0:128]/[128:300] to match the transpose chunks
                tf = work.tile([BC, H], BF16, name="tf", tag="tf")
                ti = work.tile([BC, H], BF16, name="ti", tag="ti")
                tg = work.tile([BC, H], BF16, name="tg", tag="tg")
                to = work.tile([BC, H], BF16, name="to", tag="to")
                t2_ = work.tile([BC, H], BF16, name="t2_", tag="t2_")
                t1_ = work.tile([BC, H], BF16, name="t1_", tag="t1_")
                c_new = work.tile([BC, H], BF16, name="c_new", tag="c_new")
                tc_ = work.tile([BC, H], BF16, name="tc_", tag="tc_")
                h_new = work.tile([BC, H], BF16, name="h_new", tag="h_new")

                SIG = mybir.ActivationFunctionType.Sigmoid
                TANH = mybir.ActivationFunctionType.Tanh

                f_act = nc.scalar.activation(out=tf[:], in_=psf[:], func=SIG)
                if fill_prev is not None:
                    # keep ACT FIFO order: filler(t-1) then sigmoid-f(t)
                    add_dep_helper(f_act.ins, fill_prev.ins, sync=False,
                                   reason="f after filler")
                if not last:
                    x_pass(t + 1, "f", 0, start=True)
                    x_pass(t + 1, "f", 1, start=False)
                nc.scalar.activation(out=ti[:], in_=psi[:], func=SIG)
                if not last:
                    x_pass(t + 1, "i", 0, start=True)
                    x_pass(t + 1, "i", 1, start=False)
                t2_ins = None
                if c_prev is not None:
                    t2_ins = nc.vector.tensor_mul(t2_[:], tf[:], c_prev[:])
                # previous step's capture fills the DVE window here so the
                # DVE is awake when tanh-g lands
                cap = None
                if h_prev is not None and t2_ins is not None:
                    cap = nc.vector.scalar_tensor_tensor(
                        out=hout[:], in0=h_prev[:, 0:H],
                        scalar=delta[:, t - 1 : t],
                        in1=hout[:], op0=mybir.AluOpType.mult,
                        op1=mybir.AluOpType.add,
                    )
                    add_dep_helper(cap.ins, t2_ins.ins, sync=False,
                                   reason="capture after t2")
                nc.scalar.activation(out=tg[:], in_=psg[:], func=TANH)
                if not last:
                    x_pass(t + 1, "g", 0, start=True)
                    x_pass(t + 1, "g", 1, start=False)
                # chunked t1 / c_new
                first_t1 = True
                for c0, c1 in ((0, P), (P, H)):
                    t1i = nc.vector.tensor_mul(t1_[:, c0:c1], ti[:, c0:c1],
                                               tg[:, c0:c1])
                    if first_t1 and cap is not None:
                        add_dep_helper(t1i.ins, cap.ins, sync=False,
                                       reason="t1 after capture")
                        first_t1 = False
                    if c_prev is not None:
                        nc.vector.tensor_add(c_new[:, c0:c1], t1_[:, c0:c1],
                                             t2_[:, c0:c1])
                    else:
                        nc.vector.tensor_copy(out=c_new[:, c0:c1],
                                              in_=t1_[:, c0:c1])
                nc.scalar.activation(out=to[:], in_=pso[:], func=SIG)
                nc.scalar.activation(out=tc_[:, 0:P], in_=c_new[:, 0:P],
                                     func=TANH)
                cb_act = nc.scalar.activation(out=tc_[:, P:H],
                                              in_=c_new[:, P:H], func=TANH)
                fill_prev = None

                last_copy = None
                if not last:
                    # h chunks -> separate PSUM transpose tiles -> copies
                    na, nb = htA[(t + 1) % 2], htB[(t + 1) % 2]
                    s4n = xt_slice(t + 1, 2)
                    trA = psum.tile([P, P], BF16, name="trA", tag="trpA",
                                    bufs=1)
                    trB = psum.tile([P, P], BF16, name="trB", tag="trpB",
                                    bufs=1)
                    trC = psum.tile([64, P], BF16, name="trC", tag="trpC",
                                    bufs=1)
                    nc.vector.tensor_mul(h_new[:, 0:P], to[:, 0:P], tc_[:, 0:P])
                    nc.vector.tensor_mul(h_new[:, P:2 * P], to[:, P:2 * P],
                                         tc_[:, P:2 * P])
                    nc.vector.tensor_mul(h_new[:, 2 * P:H], to[:, 2 * P:H],
                                         tc_[:, 2 * P:H])
                    nc.tensor.transpose(out=trA[:], in_=h_new[:, 0:P],
                                        identity=identb[:])
                    nc.tensor.transpose(out=trB[:], in_=h_new[:, P:2 * P],
                                        identity=identb[:])
                    nc.tensor.transpose(out=trC[0:H - 2 * P, :],
                                        in_=h_new[:, 2 * P:H],
                                        identity=identb[:])
                    nc.vector.tensor_copy(out=na[:], in_=trA[:])
                    # middle copy rides the scalar engine (it can read PSUM)
                    # so the DVE copy chain is 2 ops, not 3
                    cpb_act = nc.scalar.activation(
                        out=nb[:], in_=trB[:],
                        func=mybir.ActivationFunctionType.Copy,
                    )
                    last_copy = nc.vector.tensor_copy(
                        out=s4n[0:H - 2 * P, :],
                        in_=trC[0:H - 2 * P, :])
                    # ACT idle-window filler (see actfill above), pinned
                    # between the trB scalar-copy and sigmoid-f(t+1)
                    fill_prev = nc.scalar.activation(
                        out=actfill[:], in_=actfill[:],
                        func=mybir.ActivationFunctionType.Copy,
                    )
                    add_dep_helper(fill_prev.ins, cpb_act.ins, sync=False,
                                   reason="filler after trB copy")
                else:
                    nc.vector.tensor_mul(h_new[:, 0:P], to[:, 0:P], tc_[:, 0:P])
                    nc.vector.tensor_mul(h_new[:, P:2 * P], to[:, P:2 * P],
                                         tc_[:, P:2 * P])
                    nc.vector.tensor_mul(h_new[:, 2 * P:H], to[:, 2 * P:H],
                                         tc_[:, 2 * P:H])

                # hout += delta_t * h is deferred into the NEXT step's DVE
                # window (h_prev); the final step's capture happens after the
                # loop
                c_prev = c_new
                h_prev = h_new
                have_h = True

            # final step's capture
            nc.vector.scalar_tensor_tensor(
                out=hout[:], in0=h_prev[:, 0:H],
                scalar=delta[:, t_steps - 1 : t_steps],
                in1=hout[:], op0=mybir.AluOpType.mult,
                op1=mybir.AluOpType.add,
            )

            # ---- output projection (bf16, via the padded houtb tile)
            nc.vector.tensor_copy(out=houtb[:, 0:H], in_=hout[:])
            hot = work.tile([P, 3, P], BF16, name="hot")
            trpo = psum.tile([P, 3 * P], BF16, name="trpo", tag="trp", bufs=1)
            for k in range(3):
                nc.tensor.transpose(
                    out=trpo[:, k * P : (k + 1) * P],
                    in_=houtb[:, k * P : (k + 1) * P], identity=identb[:]
                )
            nc.vector.tensor_copy(out=hot[:, :, :], in_=trpo[:])
            po = psum.tile([P, P], F32, name="po", tag="pso", bufs=1)
            for k in range(3):
                nc.tensor.matmul(
                    out=po[:, 0:C],
                    lhsT=hot[:, k, :],
                    rhs=wout_t[k][:, :],
                    start=(k == 0),
                    stop=(k == 2),
                )
            logit = work.tile([BC, C], F32, name="logit")
            nc.vector.tensor_add(logit[:], po[:, 0:C], bout_sb[:])
            nc.sync.dma_start(out=out_e[:], in_=logit[:])

    if split_waits:
        _split_multi_waits(nc, [s.num for s in spare_sems])
    return nc


_NC_CACHE = {}


def _get_nc(t_steps=T):
    if t_steps not in _NC_CACHE:
        _NC_CACHE[t_steps] = build(t_steps)
    return _NC_CACHE[t_steps]


def make_in_maps(sent, lens, emb, Wx, Wh, b, Wout, bout):
    # permute gate columns [i|f|g|o] -> [f|i|g|o]
    perm = np.concatenate(
        [np.arange(300, 600), np.arange(0, 300), np.arange(600, 900),
         np.arange(900, 1200)]
    )
    Wxp = np.asarray(Wx, np.float32)[:, perm]
    Whp = np.asarray(Wh, np.float32)[:, perm]
    bp = np.asarray(b, np.float32)[perm]

    # combined 5-pass weights [5*128, 1200]
    wcomb = np.zeros((5 * P, G), np.float32)
    wcomb[0:P] = Wxp[0:P]
    wcomb[P:2 * P] = Wxp[P:2 * P]
    wcomb[2 * P:3 * P] = Whp[0:P]
    wcomb[3 * P:4 * P] = Whp[P:2 * P]
    # pass 4 mixed: rows 0:44 = Wh tail, 64:108 = Wx tail, 108 = bias
    wcomb[4 * P + 0 : 4 * P + (H - 2 * P)] = Whp[2 * P:H]
    wcomb[4 * P + 64 : 4 * P + 64 + (D - 2 * P)] = Wxp[2 * P:D]
    wcomb[4 * P + 64 + (D - 2 * P)] = bp
    wcomb = np.ascontiguousarray(wcomb.astype(ml_dtypes.bfloat16))

    # emb rearranged: cols 0:256 data, 256:320 zero, 320:364 tail data,
    # 364 ones, 365:384 zero
    embf = np.asarray(emb, np.float32)
    emb_pad = np.zeros((V, 3 * P), np.float32)
    emb_pad[:, 0:2 * P] = embf[:, 0:2 * P]
    emb_pad[:, 2 * P + 64 : 2 * P + 64 + (D - 2 * P)] = embf[:, 2 * P:D]
    emb_pad[:, 2 * P + 64 + (D - 2 * P)] = 1.0
    emb_pad = np.ascontiguousarray(emb_pad.astype(ml_dtypes.bfloat16))

    wout_pad = np.zeros((3 * P, C), np.float32)
    wout_pad[:H, :] = np.asarray(Wout, np.float32)
    wout = np.ascontiguousarray(wout_pad.astype(ml_dtypes.bfloat16))
    bout2 = np.asarray(bout, np.float32).reshape(1, C)
    arange = np.arange(T, dtype=np.float32).reshape(1, T)

    in_maps = []
    for i in range(N_CORES):
        sl = slice(i * BC, (i + 1) * BC)
        in_maps.append({
            "sent": np.ascontiguousarray(np.asarray(sent, np.int32)[sl]),
            "lensm1": (np.asarray(lens, np.int32)[sl] - 1).reshape(BC, 1).astype(np.float32),
            "emb": emb_pad,
            "wcomb": wcomb,
            "wout": wout,
            "bout": bout2,
            "arange": arange,
        })
    return in_maps


def kernel(sent, lens, emb, Wx, Wh, b, Wout, bout):
    nc = _get_nc(T)
    in_maps = make_in_maps(sent, lens, emb, Wx, Wh, b, Wout, bout)
    res = run_bass_kernel_spmd(nc, in_maps, core_ids=list(range(N_CORES)))
    out = np.concatenate(
        [res.results[i]["out"] for i in range(N_CORES)], axis=0
    )
    return out.astype(np.float32)


# revision 33
# speedup vs baseline: 1.0266x; 1.0015x over previous
"""Trainium2 Bass kernel for masked-LSTM sentence classifier (nn_ABSA_Lstm).

Data-parallel over 8 NeuronCores, 128 sentences per core.

v4: packed contraction. The per-step gate matmul contracts x (301 rows incl.
bias-ones) and h (300 rows) in ONE 5-pass K=620 contraction instead of 3+3
K-tiles, by host-permuting the combined weight rows:

  pass0 = x^T[  0:128]          pass1 = x^T[128:256]
  pass2 = h^T[  0:128]          pass3 = h^T[128:256]
  pass4 = [ h^T[256:300] | 0*20 | x^T[256:300] | ones | 0*19 ]  (mixed tile)

The mixed tile costs nothing extra: emb is host-rearranged so the gathered/
xbar-transposed chunk2 lands with zeros in partitions 0:64 and the x-tail in
64:128; the per-step DVE copy of the third h-transpose chunk overwrites
partitions 0:44 in place.  20 matmuls of N=300 per step (vs 24) plus 3 PE
transposes; h is transposed in three chunks (128/128/44) with three DVE
copies so the three h-passes pipeline behind the elementwise chain.

Everything else follows v3: gate order [f|i|g|o], per-gate PSUM, x-prefill
of the two pure-x passes as PE filler, indirect-DMA gather pipeline with
xbar transposes, PE warm-up, masked output via hout += delta_t * h_t, and
the multi-wait splitting post-pass.
"""

import sys

for _p in ("/opt/trn_rl_repo", "/root/.axon_site/_ro/trn_rl_repo"):
    if _p not in sys.path:
        sys.path.append(_p)

import numpy as np
import ml_dtypes

from concourse import bass, mybir
import concourse.tile as tile
from concourse.bass import IndirectOffsetOnAxis
from concourse.bass_utils import run_bass_kernel_spmd
from concourse.masks import make_identity

B, T, V, D, H, C = 1024, 80, 50000, 300, 300, 3
G = 4 * H            # 1200 gate columns, order [f | i | g | o]
N_CORES = 8
BC = B // N_CORES    # 128 sentences per core
P = 128

F32 = mybir.dt.float32
BF16 = mybir.dt.bfloat16
I32 = mybir.dt.int32

# gate column ranges in the permuted weights
NF, NI, NG, NO = (0, H), (H, 2 * H), (2 * H, 3 * H), (3 * H, G)
GATES = (NF, NI, NG, NO)


def _sync_wait(sem_id, value):
    import bass_rust
    return bass_rust.SyncWait(
        sync_type="semaphore", id=sem_id, ant_name=f"splitsem_{sem_id}",
        wait_mode="sem-ge-imm", wait_value=value, wait_reg=None,
    )


def _sync_update(sem_id, mode, value):
    import bass_rust
    return bass_rust.SyncUpdate(
        sync_type="semaphore", id=sem_id, ant_name=f"splitsem_{sem_id}",
        update_mode=mode, update_value=value, update_reg=None,
    )


def _split_multi_waits(nc, spare_sem_ids):
    """walrus caps sync waits per instruction at 1 for every struct we hit.

    Engine instructions: spill excess waits onto single-wait NoOps placed
    just before, on the same engine (engine streams are in-order).

    DMA/queue instructions: a preceding engine NoOp may not order the DGE
    ring, so the spill NoOps perform ALL the original waits and the last one
    increments a dedicated semaphore; the DMA's single wait becomes that
    semaphore. Each such semaphore is decremented back to 0 at the kernel
    tail so repeated NEFF executions stay correct."""
    f = nc.m.functions[0]
    spare = list(spare_sem_ids)
    eng_sem = {}     # engine -> sem id (one per issuing engine, in-order stream)
    eng_count = {}   # engine -> number of increments so far
    for blk in f.blocks:
        out = []
        for ins in blk.instructions:
            si = ins.sync_info
            waits = list(si.on_wait) if si and si.on_wait else []
            if len(waits) <= 1:
                out.append(ins)
                continue
            tname = type(ins).__name__
            is_dma = ("DMA" in tname or "TensorLoad" in tname
                      or "TensorSave" in tname)
            if is_dma:
                eng = ins.engine
                if eng not in eng_sem:
                    eng_sem[eng] = spare.pop()
                    eng_count[eng] = 0
                sid = eng_sem[eng]
                eng_count[eng] += 1
                target = eng_count[eng]
                for j, w in enumerate(waits):
                    nop = mybir.InstNoOp(name=f"nop-dsplit-{nc.next_id()}")
                    nop.engine = eng
                    upd = [_sync_update(sid, "sem-inc", 1)] if j == len(waits) - 1 else []
                    nop.sync_info = mybir.SyncInfo(on_wait=[w], on_update=upd)
                    out.append(nop)
                si.on_wait = [_sync_wait(sid, target)]
            else:
                for w in waits[:-1]:
                    nop = mybir.InstNoOp(name=f"nop-split-{nc.next_id()}")
                    nop.engine = ins.engine
                    nop.sync_info = mybir.SyncInfo(on_wait=[w], on_update=[])
                    out.append(nop)
                si.on_wait = waits[-1:]
            out.append(ins)
        blk.instructions = out
    # tail: restore spilled-DMA semaphores to 0 for repeat executions
    if eng_sem:
        last_blk = f.blocks[-1]
        tail = list(last_blk.instructions)
        for eng, sid in eng_sem.items():
            nop = mybir.InstNoOp(name=f"nop-dclear-{nc.next_id()}")
            nop.engine = mybir.EngineType.SP
            nop.sync_info = mybir.SyncInfo(
                on_wait=[], on_update=[_sync_update(sid, "sem-sub-imm", eng_count[eng])]
            )
            tail.append(nop)
        last_blk.instructions = tail
    return sum(eng_count.values())


def build(t_steps=T, split_waits=True):
    nc = bass.Bass()
    spare_sems = [nc.alloc_semaphore(f"splitspare{i}") for i in range(48)]

    sent_e = nc.declare_dram_parameter("sent", [BC, T], I32, isOutput=False)
    lensm1_e = nc.declare_dram_parameter("lensm1", [BC, 1], F32, isOutput=False)
    # emb host-rearranged to [V, 384]:
    #   cols 0:256   = emb[:, 0:256]
    #   cols 256:320 = 0          (chunk2 partitions 0:64 after transpose)
    #   cols 320:364 = emb[:, 256:300]
    #   col  364     = 1.0        (bias ones-row)
    #   cols 365:384 = 0
    emb_e = nc.declare_dram_parameter("emb", [V, 3 * P], BF16, isOutput=False)
    # combined per-pass weights [128, 5, 1200]
    wcomb_e = nc.declare_dram_parameter("wcomb", [5 * P, G], BF16, isOutput=False)
    wout_e = nc.declare_dram_parameter("wout", [3 * P, C], BF16, isOutput=False)
    bout_e = nc.declare_dram_parameter("bout", [1, C], F32, isOutput=False)
    arange_e = nc.declare_dram_parameter("arange", [1, T], F32, isOutput=False)
    out_e = nc.declare_dram_parameter("out", [BC, C], F32, isOutput=True)

    with tile.TileContext(nc) as tc:
        with (
            tc.tile_pool(name="const", bufs=1) as const,
            tc.tile_pool(name="wpool", bufs=1) as wpool,
            tc.tile_pool(name="xtp", bufs=1) as xtp,
            tc.tile_pool(name="work", bufs=2) as work,
            tc.tile_pool(name="psum", bufs=2, space="PSUM") as psum,
        ):
            # ---- sent first: the gather pipeline depends on it ----
            sent_sb = const.tile([BC, T], I32)
            nc.sync.dma_start(out=sent_sb[:], in_=sent_e[:])

            ident = const.tile([P, P], F32)
            identb = const.tile([P, P], BF16)
            make_identity(nc, ident)
            nc.vector.tensor_copy(out=identb[:], in_=ident[:])

            # warm the PE HAM clock gate (~3.4us of sustained activity flips
            # K=4/8 -> 8/8) with junk matmuls while the PE is otherwise idle
            # waiting for the first gathers, so the quad-0 transposes, first
            # x-matmuls, and steps 0-1 all run at full clock
            warm = psum.tile([P, P], F32, name="warm", tag="trp",
                             bufs=1)
            for _ in range(36):
                nc.tensor.matmul(out=warm[:, 0:P], lhsT=identb[:],
                                 rhs=identb[:, 0:P], start=True, stop=True)

            # x staging: one standalone tile per group, written only by
            # its gather and read only by its transpose -- zero cross-step
            # dependencies, so both DMA queues free-run ahead of the scan.
            qsizes = []
            left = t_steps
            for s in (2, 2):
                if left > 0:
                    s = min(s, left)
                    qsizes.append(s)
                    left -= s
            while left > 0:
                s = min(4, left)
                qsizes.append(s)
                left -= s
            qof = []
            for qi, s in enumerate(qsizes):
                for o in range(s):
                    qof.append((qi, o))
            xts = [
                xtp.tile([P, 3 * s, P], BF16, name=f"xt{i}")
                for i, s in enumerate(qsizes)
            ]
            xpads = [
                xtp.tile([P, s * 3 * P], BF16, name=f"xpad{i}")
                for i, s in enumerate(qsizes)
            ]

            def xt_slice(t, k):
                qi, o = qof[t]
                return xts[qi][:, 3 * o + k, :]

            def prep_gather(t):
                qi, o = qof[t]
                nc.gpsimd.indirect_dma_start(
                    out=xpads[qi][:, o * 3 * P : (o + 1) * 3 * P],
                    out_offset=None, in_=emb_e[:],
                    in_offset=IndirectOffsetOnAxis(ap=sent_sb[:, t : t + 1], axis=0),
                )

            def prep_transpose(q):
                nc.sync.dma_start_transpose(
                    out=xts[q][:, :, :], in_=xpads[q][:]
                )

            # interleave gather/transpose emission so the static schedule
            # pipelines the two queues; quad 0 is transposed on the PE (idle
            # during the prologue), skipping the xbar queue's first-hop
            # latency so step 0's x-matmuls start several us earlier
            QT = 4
            for u in range(min(QT, t_steps)):
                prep_gather(u)
                qi, o = qof[u]
                if o == qsizes[qi] - 1 and qi > 0:
                    prep_transpose(qi)
            for o in range(qsizes[0]):
                trp0 = psum.tile([P, 3 * P], BF16, name="trpq0", tag="trp",
                                 bufs=1)
                for k in range(3):
                    nc.tensor.transpose(
                        out=trp0[:, k * P : (k + 1) * P],
                        in_=xpads[0][:, (o * 3 + k) * P : (o * 3 + k + 1) * P],
                        identity=identb[:],
                    )
                nc.vector.tensor_copy(
                    out=xts[0][:, 3 * o : 3 * o + 3, :], in_=trp0[:])
            # weights on the scalar DMA queue (one DMA per tensor), in
            # parallel with the gathers and transposes
            wc_sb = wpool.tile([P, 5, G], BF16, name="wc_sb")
            wc_src = bass.AP(tensor=wcomb_e, offset=0,
                             ap=[[G, P], [P * G, 5], [1, G]])
            nc.scalar.dma_start(out=wc_sb[:], in_=wc_src)
            wc_t = [wc_sb[:, p, :] for p in range(5)]
            wout_sb = wpool.tile([P, 3, C], BF16, name="wout_sb")
            wout_src = bass.AP(tensor=wout_e, offset=0,
                               ap=[[C, P], [P * C, 3], [1, C]])
            nc.scalar.dma_start(out=wout_sb[:], in_=wout_src)
            wout_t = [wout_sb[:, k, :] for k in range(3)]

            lensm1 = const.tile([BC, 1], F32)
            nc.scalar.dma_start(out=lensm1[:], in_=lensm1_e[:])

            arange_sb = const.tile([BC, T], F32)
            arange_bcast = bass.AP(
                tensor=arange_e, offset=0, ap=[[0, BC], [1, T]]
            )
            nc.gpsimd.dma_start(out=arange_sb[:], in_=arange_bcast)

            # delta[b,t] = (t == lens[b]-1), as f32
            delta = const.tile([BC, T], F32)
            nc.vector.tensor_scalar(
                out=delta[:], in0=arange_sb[:], scalar1=lensm1[:, 0:1],
                scalar2=None, op0=mybir.AluOpType.is_equal,
            )

            bout_sb = const.tile([BC, C], F32)
            bout_bcast = bass.AP(
                tensor=bout_e, offset=0, ap=[[0, BC], [1, C]]
            )
            nc.gpsimd.dma_start(out=bout_sb[:], in_=bout_bcast)

            for u in range(QT, t_steps):
                prep_gather(u)
                qi, o = qof[u]
                if o == qsizes[qi] - 1:
                    prep_transpose(qi)

            # preload the sigmoid/tanh ACT table set during the prologue
            actpre = const.tile([BC, 1], BF16)
            nc.scalar.activation(
                out=actpre[:, 0:1], in_=lensm1[:, 0:1],
                func=mybir.ActivationFunctionType.Sigmoid,
            )

            # ---- scan state ----
            hout = const.tile([BC, H], F32)
            nc.vector.memset(hout[:], 0.0)

            # h^T rings for passes 2 and 3 (chunks 0:128 and 128:256)
            htA = [const.tile([P, P], BF16, name=f"htA{i}") for i in range(2)]
            htB = [const.tile([P, P], BF16, name=f"htB{i}") for i in range(2)]

            # padded h_new for the output projection epilogue
            houtb = const.tile([BC, 3 * P], BF16, name="houtb")
            nc.vector.memset(houtb[:, D : 3 * P], 0.0)

            # single-buffered per-gate PSUM (4 banks) + 3 transpose banks
            psf = psum.tile([BC, H], F32, name="psff", tag="psf", bufs=1)
            psi = psum.tile([BC, H], F32, name="psii", tag="psi", bufs=1)
            psg = psum.tile([BC, H], F32, name="psgg", tag="psg", bufs=1)
            pso = psum.tile([BC, H], F32, name="psoo", tag="pso", bufs=1)
            PS = {"f": psf, "i": psi, "g": psg, "o": pso}
            NR = {"f": NF, "i": NI, "g": NG, "o": NO}

            def x_pass(t, gate, p, start):
                n0, n1 = NR[gate]
                nc.tensor.matmul(
                    out=PS[gate][:, 0:H], lhsT=xt_slice(t, p),
                    rhs=wc_t[p][:, n0:n1], start=start, stop=False,
                )

            def h_pass(gate, lhsT, p, stop):
                n0, n1 = NR[gate]
                nc.tensor.matmul(
                    out=PS[gate][:, 0:H], lhsT=lhsT,
                    rhs=wc_t[p][:, n0:n1], start=False, stop=stop,
                )

            for gate in "figo":
                x_pass(0, gate, 0, start=True)
                x_pass(0, gate, 1, start=False)

            from bass_rust import add_dep_helper

            c_prev = None
            have_h = False
            for t in range(t_steps):
                last = t + 1 >= t_steps
                ra, rb = htA[t % 2], htB[t % 2]     # written at end of step t-1
                s4 = xt_slice(t, 2)                 # mixed tile (h-tail in 0:44)
                # h-passes gate-major so each gate's stop lands early
                if have_h:
                    for gate in "figo":
                        h_pass(gate, ra[:], 2, stop=False)
                        h_pass(gate, rb[:], 3, stop=False)
                        h_pass(gate, s4, 4, stop=True)
                else:
                    # step 0: x-tail still contributes through the mixed tile
                    # (partitions 0:64 are zeros from the rearranged emb)
                    for gate in "figo":
                        h_pass(gate, s4, 4, stop=True)

                # minimal-loop tail: F/I/G/O unchunked; t1/c and tanh_c
                # chunked at [0:128]/[128:300] to match the transpose chunks
                tf = work.tile([BC, H], BF16, name="tf", tag="tf")
                ti = work.tile([BC, H], BF16, name="ti", tag="ti")
                tg = work.tile([BC, H], BF16, name="tg", tag="tg")
                to = work.tile([BC, H], BF16, name="to", tag="to")
                t2_ = work.tile([BC, H], BF16, name="t2_", tag="t2_")
                t1_ = work.tile([BC, H], BF16, name="t1_", tag="t1_")
                c_new = work.tile([BC, H], BF16, name="c_new", tag="c_new")
                tc_ = work.tile([BC, H], BF16, name="tc_", tag="tc_")
                h_new = work.tile([BC, H], BF16, name="h_new", tag="h_new")

                SIG = mybir.ActivationFunctionType.Sigmoid
                TANH = mybir.ActivationFunctionType.Tanh

                nc.scalar.activation(out=tf[:], in_=psf[:], func=SIG)
                if not last:
                    x_pass(t + 1, "f", 0, start=True)
                    x_pass(t + 1, "f", 1, start=False)
                nc.scalar.activation(out=ti[:], in_=psi[:], func=SIG)
                if not last:
                    x_pass(t + 1, "i", 0, start=True)
                    x_pass(t + 1, "i", 1, start=False)
                if c_prev is not None:
                    nc.vector.tensor_mul(t2_[:], tf[:], c_prev[:])
                nc.scalar.activation(out=tg[:], in_=psg[:], func=TANH)
                if not last:
                    x_pass(t + 1, "g", 0, start=True)
                    x_pass(t + 1, "g", 1, start=False)
                # chunked t1 / c_new
                for c0, c1 in ((0, P), (P, H)):
                    nc.vector.tensor_mul(t1_[:, c0:c1], ti[:, c0:c1],
                                         tg[:, c0:c1])
                    if c_prev is not None:
                        nc.vector.tensor_add(c_new[:, c0:c1], t1_[:, c0:c1],
                                             t2_[:, c0:c1])
                    else:
                        nc.vector.tensor_copy(out=c_new[:, c0:c1],
                                              in_=t1_[:, c0:c1])
                nc.scalar.activation(out=to[:], in_=pso[:], func=SIG)
                nc.scalar.activation(out=tc_[:, 0:P], in_=c_new[:, 0:P],
                                     func=TANH)
                nc.scalar.activation(out=tc_[:, P:H], in_=c_new[:, P:H],
                                     func=TANH)

                last_copy = None
                if not last:
                    # h chunks -> separate PSUM transpose tiles -> copies
                    na, nb = htA[(t + 1) % 2], htB[(t + 1) % 2]
                    s4n = xt_slice(t + 1, 2)
                    trA = psum.tile([P, P], BF16, name="trA", tag="trpA",
                                    bufs=1)
                    trB = psum.tile([P, P], BF16, name="trB", tag="trpB",
                                    bufs=1)
                    trC = psum.tile([64, P], BF16, name="trC", tag="trpC",
                                    bufs=1)
                    nc.vector.tensor_mul(h_new[:, 0:P], to[:, 0:P], tc_[:, 0:P])
                    nc.vector.tensor_mul(h_new[:, P:2 * P], to[:, P:2 * P],
                                         tc_[:, P:2 * P])
                    nc.vector.tensor_mul(h_new[:, 2 * P:H], to[:, 2 * P:H],
                                         tc_[:, 2 * P:H])
                    nc.tensor.transpose(out=trA[:], in_=h_new[:, 0:P],
                                        identity=identb[:])
                    nc.tensor.transpose(out=trB[:], in_=h_new[:, P:2 * P],
                                        identity=identb[:])
                    nc.tensor.transpose(out=trC[0:H - 2 * P, :],
                                        in_=h_new[:, 2 * P:H],
                                        identity=identb[:])
                    nc.vector.tensor_copy(out=na[:], in_=trA[:])
                    nc.vector.tensor_copy(out=nb[:], in_=trB[:])
                    last_copy = nc.vector.tensor_copy(
                        out=s4n[0:H - 2 * P, :],
                        in_=trC[0:H - 2 * P, :])
                    # o's x-prefill after the transposes in the PE stream
                    x_pass(t + 1, "o", 0, start=True)
                    x_pass(t + 1, "o", 1, start=False)
                else:
                    nc.vector.tensor_mul(h_new[:, 0:P], to[:, 0:P], tc_[:, 0:P])
                    nc.vector.tensor_mul(h_new[:, P:2 * P], to[:, P:2 * P],
                                         tc_[:, P:2 * P])
                    nc.vector.tensor_mul(h_new[:, 2 * P:H], to[:, 2 * P:H],
                                         tc_[:, 2 * P:H])

                # hout += delta_t * h  (off the critical chain; pinned after
                # the last ht copy so it can't steal the DVE slot before it)
                cap = nc.vector.scalar_tensor_tensor(
                    out=hout[:], in0=h_new[:, 0:H], scalar=delta[:, t : t + 1],
                    in1=hout[:], op0=mybir.AluOpType.mult, op1=mybir.AluOpType.add,
                )
                if last_copy is not None:
                    add_dep_helper(cap.ins, last_copy.ins, sync=False,
                                   reason="capture after ht copy")
                c_prev = c_new
                have_h = True

            # ---- output projection (bf16, via the padded houtb tile)
            nc.vector.tensor_copy(out=houtb[:, 0:H], in_=hout[:])
            hot = work.tile([P, 3, P], BF16, name="hot")
            trpo = psum.tile([P, 3 * P], BF16, name="trpo", tag="trp", bufs=1)
            for k in range(3):
                nc.tensor.transpose(
                    out=trpo[:, k * P : (k + 1) * P],
                    in_=houtb[:, k * P : (k + 1) * P], identity=identb[:]
                )
            nc.vector.tensor_copy(out=hot[:, :, :], in_=trpo[:])
            po = psum.tile([P, P], F32, name="po", tag="pso", bufs=1)
            for k in range(3):
                nc.tensor.matmul(
                    out=po[:, 0:C],
                    lhsT=hot[:, k, :],
                    rhs=wout_t[k][:, :],
                    start=(k == 0),
                    stop=(k == 2),
                )
            logit = work.tile([BC, C], F32, name="logit")
            nc.vector.tensor_add(logit[:], po[:, 0:C], bout_sb[:])
            nc.sync.dma_start(out=out_e[:], in_=logit[:])

    if split_waits:
        _split_multi_waits(nc, [s.num for s in spare_sems])
    return nc


_NC_CACHE = {}


def _get_nc(t_steps=T):
    if t_steps not in _NC_CACHE:
        _NC_CACHE[t_steps] = build(t_steps)
    return _NC_CACHE[t_steps]


def make_in_maps(sent, lens, emb, Wx, Wh, b, Wout, bout):
    # permute gate columns [i|f|g|o] -> [f|i|g|o]
    perm = np.concatenate(
        [np.arange(300, 600), np.arange(0, 300), np.arange(600, 900),
         np.arange(900, 1200)]
    )
    Wxp = np.asarray(Wx, np.float32)[:, perm]
    Whp = np.asarray(Wh, np.float32)[:, perm]
    bp = np.asarray(b, np.float32)[perm]

    # combined 5-pass weights [5*128, 1200]
    wcomb = np.zeros((5 * P, G), np.float32)
    wcomb[0:P] = Wxp[0:P]
    wcomb[P:2 * P] = Wxp[P:2 * P]
    wcomb[2 * P:3 * P] = Whp[0:P]
    wcomb[3 * P:4 * P] = Whp[P:2 * P]
    # pass 4 mixed: rows 0:44 = Wh tail, 64:108 = Wx tail, 108 = bias
    wcomb[4 * P + 0 : 4 * P + (H - 2 * P)] = Whp[2 * P:H]
    wcomb[4 * P + 64 : 4 * P + 64 + (D - 2 * P)] = Wxp[2 * P:D]
    wcomb[4 * P + 64 + (D - 2 * P)] = bp
    wcomb = np.ascontiguousarray(wcomb.astype(ml_dtypes.bfloat16))

    # emb rearranged: cols 0:256 data, 256:320 zero, 320:364 tail data,
    # 364 ones, 365:384 zero
    embf = np.asarray(emb, np.float32)
    emb_pad = np.zeros((V, 3 * P), np.float32)
    emb_pad[:, 0:2 * P] = embf[:, 0:2 * P]
    emb_pad[:, 2 * P + 64 : 2 * P + 64 + (D - 2 * P)] = embf[:, 2 * P:D]
    emb_pad[:, 2 * P + 64 + (D - 2 * P)] = 1.0
    emb_pad = np.ascontiguousarray(emb_pad.astype(ml_dtypes.bfloat16))

    wout_pad = np.zeros((3 * P, C), np.float32)
    wout_pad[:H, :] = np.asarray(Wout, np.float32)
    wout = np.ascontiguousarray(wout_pad.astype(ml_dtypes.bfloat16))
    bout2 = np.asarray(bout, np.float32).reshape(1, C)
    arange = np.arange(T, dtype=np.float32).reshape(1, T)

    in_maps = []
    for i in range(N_CORES):
        sl = slice(i * BC, (i + 1) * BC)
        in_maps.append({
            "sent": np.ascontiguousarray(np.asarray(sent, np.int32)[sl]),
            "lensm1": (np.asarray(lens, np.int32)[sl] - 1).reshape(BC, 1).astype(np.float32),
            "emb": emb_pad,
            "wcomb": wcomb,
            "wout": wout,
            "bout": bout2,
            "arange": arange,
        })
    return in_maps


def kernel(sent, lens, emb, Wx, Wh, b, Wout, bout):
    nc = _get_nc(T)
    in_maps = make_in_maps(sent, lens, emb, Wx, Wh, b, Wout, bout)
    res = run_bass_kernel_spmd(nc, in_maps, core_ids=list(range(N_CORES)))
    out = np.concatenate(
        [res.results[i]["out"] for i in range(N_CORES)], axis=0
    )
    return out.astype(np.float32)
